# revision 3
# baseline (speedup 1.0000x reference)
"""Trainium2 Bass kernel for a 2-layer GCN + 2 FC layers (nn_CGNN_more_fc).

Strategy (8 NeuronCores, SPMD):
  - Nodes partitioned contiguously across cores (6250/core, padded to 6272).
  - Edges partitioned by destination owner; host sorts edges by
    (dst tile, src-table-half, src) and pads per-group to 128-slot chunks.
    Explicit self-loop slots are appended so the (A+I) term is one matmul.
  - GCN algebra is reordered so each conv aggregates 128-dim features:
      conv1: agg(x) @ W1   (aggregate-then-transform, d_in=128 < d_out=512)
      conv2: agg(h1 @ W3)  (transform-then-aggregate, d_out=128 < d_in=512)
    with agg(z) = diag(dinv) (S + I) diag(dinv) z, dinv = 1/sqrt(deg).
  - Per conv: each core scales its node block by dinv (pre-scale), an
    AllGather builds the full bf16 node table in every core's HBM, then
    dma_gather (GPSIMD SWDGE) pulls source rows chunk-by-chunk, a 0/1
    one-hot built on the vector engine (broadcast-AP is_equal vs iota)
    scatter-reduces them on the tensor engine into PSUM per dst tile
    (transposed layout: [feat x dst]), and the dst-side dinv post-scale is
    applied on evacuation.
  - Dense layers run in feature-on-partition layout so biases fold into
    per-partition activation ops; leaky relu is max(v, 0.01 v) on DVE.
  - int16 gather indices can't span 50176 rows, so each batch issues two
    gathers: one against the table base (rows < 32768) and one against a
    +32768-row slice.

Host-side numpy does only integer graph restructuring (partition, sort,
pad, degree counts as a byproduct of sorting); every FLOP of the reference
computation runs on device.
"""
import numpy as np
import ml_dtypes
from contextlib import ExitStack
from dataclasses import dataclass

from concourse import bass, bacc, mybir, tile
from concourse import bass_utils

DT = mybir.dt
BF16 = ml_dtypes.bfloat16
HALF = 32768


@dataclass(frozen=True)
class Cfg:
    n_nodes: int = 50000
    n_cores: int = 8
    feat: int = 128          # node feature dim (must be 128)
    hid: int = 512           # conv1 output dim
    fc1: int = 256           # fc1 output dim
    slot_batch: int = 9216   # max gather slots per batch

    @property
    def nloc(self):
        return self.n_nodes // self.n_cores

    @property
    def ntile(self):
        return (self.nloc + 127) // 128

    @property
    def npad(self):
        return self.ntile * 128

    @property
    def trows(self):
        return self.npad * self.n_cores


@dataclass
class Meta:
    """Common (cross-core) slot structure + per-core host arrays."""
    lo_ch: np.ndarray        # [NT] chunks in low group per tile
    hi_ch: np.ndarray        # [NT] chunks in high group per tile
    batches: list            # list of list-of-tile-indices
    ctot: int                # total chunks
    idx_w: list              # per-core wrapped int16 [128, slots//16]
    lab_w: list              # per-core [128, ctot] bf16


def _prep(cfg: Cfg, src, dst):
    """Partition + sort edges; build the common padded slot layout."""
    NLOC, NT, NPAD = cfg.nloc, cfg.ntile, cfg.npad
    owner = dst // NLOC
    dlocal = dst - owner * NLOC
    srow = (src // NLOC) * NPAD + (src % NLOC)

    per_core = []
    for c in range(cfg.n_cores):
        sel = owner == c
        es = np.concatenate([srow[sel], c * NPAD + np.arange(NLOC, dtype=np.int64)])
        ed = np.concatenate([dlocal[sel], np.arange(NLOC, dtype=np.int64)])
        tid = ed >> 7
        lab = (ed & 127).astype(np.float32)
        hi = (es >= HALF).astype(np.int64)
        order = np.lexsort((es, hi, tid))
        per_core.append((es[order], lab[order], tid[order], hi[order]))

    # common chunk counts per tile/group = max over cores
    lo_ch = np.zeros(NT, np.int64)
    hi_ch = np.zeros(NT, np.int64)
    for es, lab, tid, hi in per_core:
        for t in range(NT):
            m = tid == t
            nlo = int(np.count_nonzero(m & (hi == 0)))
            nhi = int(np.count_nonzero(m & (hi == 1)))
            lo_ch[t] = max(lo_ch[t], (nlo + 127) // 128)
            hi_ch[t] = max(hi_ch[t], (nhi + 127) // 128)

    # batches: greedy groups of tiles
    batches, cur, cur_slots = [], [], 0
    for t in range(NT):
        ts = int(lo_ch[t] + hi_ch[t]) * 128
        if cur and cur_slots + ts > cfg.slot_batch:
            batches.append(cur)
            cur, cur_slots = [], 0
        cur.append(t)
        cur_slots += ts
    if cur:
        batches.append(cur)

    ctot = int((lo_ch + hi_ch).sum())
    tot_slots = ctot * 128

    # per-core slot streams in [batch: lo segs..., hi segs...] order
    idx_w_all, lab_w_all = [], []
    for es, lab, tid, hi in per_core:
        idx_s = np.zeros(tot_slots, np.int16)
        lab_s = np.full(tot_slots, -1.0, np.float32)
        pos = 0
        for tiles in batches:
            for grp in (0, 1):
                for t in tiles:
                    m = (tid == t) & (hi == grp)
                    e = es[m] - (HALF if grp else 0)
                    l = lab[m]
                    nch = int(lo_ch[t] if grp == 0 else hi_ch[t])
                    idx_s[pos:pos + len(e)] = e.astype(np.int16)
                    lab_s[pos:pos + len(l)] = l
                    pos += nch * 128
        assert pos == tot_slots
        # wrap idx: slot i -> [(i%16)+16r, i//16]
        iw = np.tile(idx_s.reshape(-1, 16).T, (8, 1)).copy()
        lw = lab_s.reshape(-1, 128).T.astype(BF16).copy()
        idx_w_all.append(iw)
        lab_w_all.append(lw)

    return Meta(lo_ch=lo_ch, hi_ch=hi_ch, batches=batches, ctot=ctot,
                idx_w=idx_w_all, lab_w=lab_w_all)


def _leaky(nc, out_ap, in_ap):
    nc.vector.scalar_tensor_tensor(out_ap, in_ap, 0.01, in_ap,
                                   mybir.AluOpType.mult, mybir.AluOpType.max)


def _dense_chunks(npad):
    """(offset, width) n-chunks of <=512 columns."""
    out = []
    o = 0
    while o < npad:
        w = min(512, npad - o)
        out.append((o, w))
        o += w
    return out


def build(cfg: Cfg, meta: Meta, enable_asserts=False):
    """Build + compile the SPMD program (same NEFF for all cores)."""
    NT, NPAD, TROWS, F = cfg.ntile, cfg.npad, cfg.trows, cfg.feat
    HID, FC1 = cfg.hid, cfg.fc1
    HID_T, FC1_T = HID // 128, FC1 // 128
    tot16 = meta.ctot * 128 // 16

    nc = bacc.Bacc("TRN2", target_bir_lowering=False, debug=False,
                   num_devices=cfg.n_cores, enable_asserts=enable_asserts,
                   num_swdge_queues=4)

    x_in = nc.dram_tensor("x_in", (NPAD, F), DT.float32, kind="ExternalInput")
    degc_in = nc.dram_tensor("degc_in", (128, NT), DT.float32, kind="ExternalInput")
    idx_in = nc.dram_tensor("idx_in", (128, tot16), DT.int16, kind="ExternalInput")
    lab_in = nc.dram_tensor("lab_in", (128, meta.ctot), DT.bfloat16, kind="ExternalInput")
    iota_in = nc.dram_tensor("iota_in", (128, 128), DT.bfloat16, kind="ExternalInput")
    id_in = nc.dram_tensor("id_in", (128, 128), DT.bfloat16, kind="ExternalInput")
    idf_in = nc.dram_tensor("idf_in", (128, 128), DT.float32, kind="ExternalInput")
    W1_in = nc.dram_tensor("W1_in", (F, HID), DT.float32, kind="ExternalInput")
    b1_in = nc.dram_tensor("b1_in", (HID,), DT.float32, kind="ExternalInput")
    W3_in = nc.dram_tensor("W3_in", (HID, F), DT.float32, kind="ExternalInput")
    b3_in = nc.dram_tensor("b3_in", (F,), DT.float32, kind="ExternalInput")
    f1w_in = nc.dram_tensor("f1w_in", (FC1, F), DT.float32, kind="ExternalInput")
    f1b_in = nc.dram_tensor("f1b_in", (FC1,), DT.float32, kind="ExternalInput")
    f2w_in = nc.dram_tensor("f2w_in", (1, FC1), DT.float32, kind="ExternalInput")
    f2b_in = nc.dram_tensor("f2b_in", (1,), DT.float32, kind="ExternalInput")
    out_d = nc.dram_tensor("out", (cfg.nloc,), DT.float32, kind="ExternalOutput")

    nchunks = _dense_chunks(NPAD)

    with tile.TileContext(nc) as tc:
        with ExitStack() as top:
            perm = top.enter_context(tc.tile_pool(name="perm", bufs=1))
            dram = top.enter_context(tc.tile_pool(name="dram", bufs=1, space="DRAM"))

            # ---------- constants / weights ----------
            iota_t = perm.tile([128, 128], DT.bfloat16)
            nc.sync.dma_start(iota_t[:], iota_in.ap())
            ident_t = perm.tile([128, 128], DT.bfloat16)
            nc.sync.dma_start(ident_t[:], id_in.ap())
            identf_t = perm.tile([128, 128], DT.float32)
            nc.sync.dma_start(identf_t[:], idf_in.ap())
            idx_t = perm.tile([128, tot16], DT.int16)
            nc.sync.dma_start(idx_t[:], idx_in.ap())
            lab_t = perm.tile([128, meta.ctot], DT.bfloat16)
            nc.sync.dma_start(lab_t[:], lab_in.ap())

            w1b = perm.tile([128, HID], DT.bfloat16)
            w3b = perm.tile([128, HID_T, F], DT.bfloat16)
            b1t = perm.tile([128, HID_T], DT.float32)
            b3t = perm.tile([128, 1], DT.float32)
            f1bt = perm.tile([128, FC1_T], DT.float32)
            f2bt = perm.tile([1, 1], DT.float32)
            f1wT = perm.tile([128, FC1_T, 128], DT.bfloat16)
            f2wT = perm.tile([128, FC1_T], DT.bfloat16)

            with tc.tile_pool(name="wprep", bufs=2) as wprep, \
                 tc.tile_pool(name="wprep_ps", bufs=2, space="PSUM") as wprep_ps:
                w1f = wprep.tile([128, HID], DT.float32, tag="w1stage")
                nc.sync.dma_start(w1f[:], W1_in.ap())
                nc.vector.tensor_copy(w1b[:], w1f[:])
                for k in range(HID_T):
                    w3f = wprep.tile([128, F], DT.float32, tag="w3stage")
                    nc.sync.dma_start(w3f[:], W3_in.ap()[k * 128:(k + 1) * 128, :])
                    nc.vector.tensor_copy(w3b[:, k, :], w3f[:])
                nc.sync.dma_start(b1t[:], b1_in.ap().rearrange("(c p) -> p c", p=128))
                nc.sync.dma_start(b3t[:], b3_in.ap().rearrange("(p) -> p ()"))
                nc.sync.dma_start(f1bt[:], f1b_in.ap().rearrange("(c p) -> p c", p=128))
                nc.sync.dma_start(f2bt[:], f2b_in.ap().rearrange("(u) -> u ()"))
                for k in range(FC1_T):
                    wf = wprep.tile([128, 128], DT.float32, tag="f1stage")
                    nc.sync.dma_start(wf[:], f1w_in.ap()[k * 128:(k + 1) * 128, :])
                    wb = wprep.tile([128, 128], DT.bfloat16, tag="f1stageb")
                    nc.vector.tensor_copy(wb[:], wf[:])
                    pt = wprep_ps.tile([128, 128], DT.bfloat16)
                    nc.tensor.transpose(pt[:], wb[:], ident_t[:])
                    nc.scalar.copy(f1wT[:, k, :], pt[:])
                    # fc2_w chunk k -> [128, 1]
                    wf2 = wprep.tile([128, 1], DT.float32, tag="f2stage")
                    nc.sync.dma_start(
                        wf2[:],
                        f2w_in.ap()[0:1, k * 128:(k + 1) * 128].rearrange("u (p v) -> (p u) v", p=128))
                    nc.vector.tensor_copy(f2wT[:, k:k + 1], wf2[:])

                # ---------- dinv ----------
                degc = wprep.tile([128, 128], DT.float32, tag="degc")
                nc.vector.memset(degc[:], 1.0)
                nc.sync.dma_start(degc[:, :NT], degc_in.ap())
                dinv_c = perm.tile([128, NT], DT.float32)
                rec = wprep.tile([128, 128], DT.float32, tag="rec")
                nc.vector.reciprocal(rec[:], degc[:])
                nc.scalar.activation(rec[:], rec[:], mybir.ActivationFunctionType.Sqrt)
                nc.vector.tensor_copy(dinv_c[:], rec[:, :NT])
                # transpose -> rows, ship flat to DRAM, broadcast-load
                ptd = wprep_ps.tile([128, 128], DT.float32, tag="ptd")
                nc.tensor.transpose(ptd[:], rec[:], identf_t[:])
                rows = wprep.tile([128, 128], DT.float32, tag="rows")
                nc.scalar.copy(rows[:], ptd[:])
                dflat = dram.tile([NT, 128], DT.float32)
                nc.sync.dma_start(dflat[:], rows[:NT, :])

            dinv_b = perm.tile([128, NPAD], DT.float32)
            nc.sync.dma_start(
                dinv_b[:],
                dflat[:].rearrange("t f -> () (t f)").broadcast_to([128, NPAD]))

            # ---------- tables ----------
            xs_loc = dram.tile([NPAD, F], DT.bfloat16)
            xs_tab = dram.tile([TROWS, F], DT.bfloat16, addr_space="Shared")
            zs_loc = dram.tile([NPAD, F], DT.bfloat16)
            zs_tab = dram.tile([TROWS, F], DT.bfloat16, addr_space="Shared")

            # phase A: xs_loc = dinv * x  (row layout)
            with tc.tile_pool(name="xprep", bufs=3) as xprep:
                for t in range(NT):
                    xt = xprep.tile([128, F], DT.float32, tag="xstage")
                    nc.sync.dma_start(xt[:], x_in.ap()[t * 128:(t + 1) * 128, :])
                    xst = xprep.tile([128, F], DT.bfloat16, tag="xsstage")
                    nc.scalar.activation(xst[:], xt[:], mybir.ActivationFunctionType.Copy,
                                         scale=dinv_c[:, t:t + 1])
                    nc.sync.dma_start(xs_loc[t * 128:(t + 1) * 128, :], xst[:])
            nc.gpsimd.collective_compute(
                "AllGather", mybir.AluOpType.bypass,
                replica_groups=[list(range(cfg.n_cores))],
                ins=[xs_loc.opt()], outs=[xs_tab.opt()])

            # ---------- aggregation driver ----------
            gq = [0]

            def aggregate(table, evac):
                """Gather+scatter-matmul all batches; evac(t, psum_ap) consumes
                the [feat x dst] fp32 accumulation of tile t."""
                with tc.tile_pool(name="gpool", bufs=2) as gpool, \
                     tc.tile_pool(name="opool", bufs=2) as opool, \
                     tc.tile_pool(name="agg_ps", bufs=4, space="PSUM") as agg_ps:
                    c0 = 0
                    for tiles in meta.batches:
                        lo_b = int(sum(meta.lo_ch[t] for t in tiles))
                        hi_b = int(sum(meta.hi_ch[t] for t in tiles))
                        c_b = lo_b + hi_b
                        gbuf = gpool.tile([128, c_b, F], DT.bfloat16, tag="gbuf")
                        s_lo = c0 * 128
                        s_hi = (c0 + lo_b) * 128
                        if lo_b:
                            nc.gpsimd.dma_gather(
                                gbuf[:, 0:lo_b, :], table[:],
                                idx_t[:, s_lo // 16:(s_lo + lo_b * 128) // 16],
                                num_idxs=lo_b * 128, num_idxs_reg=lo_b * 128,
                                elem_size=F, queue_num=gq[0] % 4,
                                single_packet=False)
                            gq[0] += 1
                        if hi_b:
                            nc.gpsimd.dma_gather(
                                gbuf[:, lo_b:c_b, :], table[HALF:, :],
                                idx_t[:, s_hi // 16:(s_hi + hi_b * 128) // 16],
                                num_idxs=hi_b * 128, num_idxs_reg=hi_b * 128,
                                elem_size=F, queue_num=gq[0] % 4,
                                single_packet=False)
                            gq[0] += 1
                        oh = opool.tile([128, c_b, 128], DT.bfloat16, tag="oh")
                        nc.vector.tensor_tensor(
                            oh[:],
                            lab_t[:, c0:c0 + c_b].broadcast_to([128, c_b, 128]),
                            iota_t[:].rearrange("p d -> p () d").broadcast_to([128, c_b, 128]),
                            mybir.AluOpType.is_equal)
                        lo_off, hi_off = 0, lo_b
                        for t in tiles:
                            chunks = ([lo_off + j for j in range(int(meta.lo_ch[t]))]
                                      + [hi_off + j for j in range(int(meta.hi_ch[t]))])
                            pt = agg_ps.tile([128, 128], DT.float32, tag="aggps")
                            for k, j in enumerate(chunks):
                                nc.tensor.matmul(pt[:], lhsT=gbuf[:, j, :], rhs=oh[:, j, :],
                                                 start=(k == 0), stop=(k == len(chunks) - 1))
                            evac(t, pt)
                            lo_off += int(meta.lo_ch[t])
                            hi_off += int(meta.hi_ch[t])
                        c0 += c_b

            # ---------- conv1 + z table ----------
            with ExitStack() as s1:
                h1pool = s1.enter_context(tc.tile_pool(name="h1pool", bufs=1))
                h1T = h1pool.tile([128, HID_T, NPAD], DT.bfloat16)

                with ExitStack() as s0:
                    c1pool = s0.enter_context(tc.tile_pool(name="c1pool", bufs=1))
                    a1T = c1pool.tile([128, NPAD], DT.bfloat16)

                    def evac1(t, pt):
                        nc.vector.tensor_mul(a1T[:, t * 128:(t + 1) * 128], pt[:],
                                             dinv_b[:, t * 128:(t + 1) * 128])

                    aggregate(xs_tab, evac1)

                    # h1T[fo] = leaky(a1T.T @ W1 + b1) in [fo x n] layout
                    with tc.tile_pool(name="c1ps", bufs=3, space="PSUM") as c1ps, \
                         tc.tile_pool(name="c1tmp", bufs=3) as c1tmp:
                        for (o, w) in nchunks:
                            for fo in range(HID_T):
                                pd = c1ps.tile([128, 512], DT.float32, tag="c1ps")
                                nc.tensor.matmul(pd[:, :w], lhsT=w1b[:, fo * 128:(fo + 1) * 128],
                                                 rhs=a1T[:, o:o + w], start=True, stop=True)
                                tmp = c1tmp.tile([128, 512], DT.float32, tag="c1t")
                                nc.scalar.activation(tmp[:, :w], pd[:, :w],
                                                     mybir.ActivationFunctionType.Identity,
                                                     bias=b1t[:, fo:fo + 1])
                                _leaky(nc, h1T[:, fo, o:o + w], tmp[:, :w])

                # z2 -> zs table (scaled, transposed to row layout)
                with tc.tile_pool(name="zpool", bufs=1) as zpool, \
                     tc.tile_pool(name="zps", bufs=3, space="PSUM") as zps:
                    zsT = zpool.tile([128, NPAD], DT.bfloat16)
                    for (o, w) in nchunks:
                        pz = zps.tile([128, 512], DT.float32, tag="zps")
                        for k in range(HID_T):
                            nc.tensor.matmul(pz[:, :w], lhsT=w3b[:, k, :],
                                             rhs=h1T[:, k, o:o + w],
                                             start=(k == 0), stop=(k == HID_T - 1))
                        nc.vector.tensor_mul(zsT[:, o:o + w], pz[:, :w], dinv_b[:, o:o + w])
                    with tc.tile_pool(name="trps", bufs=3, space="PSUM") as trps, \
                         tc.tile_pool(name="trsb", bufs=3) as trsb:
                        for t in range(NT):
                            ptr = trps.tile([128, 128], DT.bfloat16, tag="trp")
                            nc.tensor.transpose(ptr[:], zsT[:, t * 128:(t + 1) * 128], ident_t[:])
                            row = trsb.tile([128, F], DT.bfloat16, tag="trs")
                            nc.scalar.copy(row[:], ptr[:])
                            nc.sync.dma_start(zs_loc[t * 128:(t + 1) * 128, :], row[:])
            nc.gpsimd.collective_compute(
                "AllGather", mybir.AluOpType.bypass,
                replica_groups=[list(range(cfg.n_cores))],
                ins=[zs_loc.opt()], outs=[zs_tab.opt()])

            # ---------- conv2 / fc ----------
            with ExitStack() as s2:
                h2pool = s2.enter_context(tc.tile_pool(name="h2pool", bufs=1))
                h2T = h2pool.tile([128, NPAD], DT.bfloat16)
                with tc.tile_pool(name="e_tmp", bufs=4) as e_tmp:

                    def evac2(t, pt):
                        sl = slice(t * 128, (t + 1) * 128)
                        v = e_tmp.tile([128, 128], DT.float32, tag="ev")
                        nc.vector.tensor_mul(v[:], pt[:], dinv_b[:, sl])
                        v2 = e_tmp.tile([128, 128], DT.float32, tag="ev2")
                        nc.scalar.activation(v2[:], v[:],
                                             mybir.ActivationFunctionType.Identity,
                                             bias=b3t[:])
                        _leaky(nc, h2T[:, sl], v2[:])

                    aggregate(zs_tab, evac2)

                with ExitStack() as s3:
                    h3pool = s3.enter_context(tc.tile_pool(name="h3pool", bufs=1))
                    h3T = h3pool.tile([128, FC1_T, NPAD], DT.bfloat16)
                    with tc.tile_pool(name="fps", bufs=3, space="PSUM") as fps, \
                         tc.tile_pool(name="ftmp", bufs=3) as ftmp:
                        for (o, w) in nchunks:
                            for k in range(FC1_T):
                                pf = fps.tile([128, 512], DT.float32, tag="fps")
                                nc.tensor.matmul(pf[:, :w], lhsT=f1wT[:, k, :],
                                                 rhs=h2T[:, o:o + w], start=True, stop=True)
                                tmp = ftmp.tile([128, 512], DT.float32, tag="ft")
                                nc.scalar.activation(tmp[:, :w], pf[:, :w],
                                                     mybir.ActivationFunctionType.Identity,
                                                     bias=f1bt[:, k:k + 1])
                                _leaky(nc, h3T[:, k, o:o + w], tmp[:, :w])

                    out_sb = h3pool.tile([1, NPAD], DT.float32)
                    with tc.tile_pool(name="gps", bufs=2, space="PSUM") as gps:
                        for (o, w) in nchunks:
                            pg = gps.tile([1, 512], DT.float32, tag="gps")
                            for k in range(FC1_T):
                                nc.tensor.matmul(pg[:, :w], lhsT=f2wT[:, k:k + 1],
                                                 rhs=h3T[:, k, o:o + w],
                                                 start=(k == 0), stop=(k == FC1_T - 1))
                            nc.scalar.activation(out_sb[:, o:o + w], pg[:, :w],
                                                 mybir.ActivationFunctionType.Identity,
                                                 bias=f2bt[:])
                    nc.sync.dma_start(out_d.ap().rearrange("(n) -> () n"), out_sb[:, :cfg.nloc])

    nc.compile()
    return nc


def make_in_maps(cfg: Cfg, meta: Meta, x, deg, W1, b1, W3, b3, fc1_w, fc1_b, fc2_w, fc2_b):
    NLOC, NT, NPAD, F = cfg.nloc, cfg.ntile, cfg.npad, cfg.feat
    iota_np = np.broadcast_to(np.arange(128, dtype=np.float32)[None, :], (128, 128)).astype(BF16).copy()
    ident_np = np.eye(128, dtype=np.float32)
    shared = dict(
        iota_in=iota_np, id_in=ident_np.astype(BF16), idf_in=ident_np,
        W1_in=np.ascontiguousarray(W1, np.float32),
        b1_in=np.ascontiguousarray(b1, np.float32),
        W3_in=np.ascontiguousarray(W3, np.float32),
        b3_in=np.ascontiguousarray(b3, np.float32),
        f1w_in=np.ascontiguousarray(fc1_w, np.float32),
        f1b_in=np.ascontiguousarray(fc1_b, np.float32),
        f2w_in=np.ascontiguousarray(fc2_w, np.float32).reshape(1, cfg.fc1),
        f2b_in=np.ascontiguousarray(fc2_b, np.float32).reshape(1),
    )
    in_maps = []
    for c in range(cfg.n_cores):
        xc = np.zeros((NPAD, F), np.float32)
        xc[:NLOC] = x[c * NLOC:(c + 1) * NLOC]
        dc = np.ones(NPAD, np.float32)
        dc[:NLOC] = deg[c * NLOC:(c + 1) * NLOC]
        degc = dc.reshape(NT, 128).T.copy()
        in_maps.append(dict(shared, x_in=xc, degc_in=degc,
                            idx_in=meta.idx_w[c], lab_in=meta.lab_w[c]))
    return in_maps


_BUILD_CACHE = {}


def kernel(x, edge_index, W1, b1, W3, b3, fc1_w, fc1_b, fc2_w, fc2_b):
    cfg = Cfg()
    x = np.asarray(x, np.float32)
    ei = np.asarray(edge_index)
    src = ei[0].astype(np.int64)
    dst = ei[1].astype(np.int64)
    deg = (np.bincount(dst, minlength=cfg.n_nodes) + 1).astype(np.float32)

    key = hash(ei.tobytes())
    if key not in _BUILD_CACHE:
        meta = _prep(cfg, src, dst)
        nc = build(cfg, meta)
        _BUILD_CACHE[key] = (meta, nc)
    meta, nc = _BUILD_CACHE[key]

    in_maps = make_in_maps(cfg, meta, x, deg, W1, b1, W3, b3, fc1_w, fc1_b, fc2_w, fc2_b)
    res = bass_utils.run_bass_kernel_spmd(nc, in_maps, core_ids=list(range(cfg.n_cores)))
    out = np.concatenate([res.results[c]["out"] for c in range(cfg.n_cores)])
    return out.astype(np.float32)


# revision 5
# speedup vs baseline: 1.2364x; 1.2364x over previous
"""Trainium2 Bass kernel for a 2-layer GCN + 2 FC layers (nn_CGNN_more_fc).

Strategy (8 NeuronCores, SPMD):
  - Nodes partitioned contiguously across cores (6250/core, padded to 6272).
  - Edges partitioned by destination owner; host sorts edges by
    (dst tile, src-table-half, src) and pads per-group to 128-slot chunks.
    Explicit self-loop slots are appended so the (A+I) term is one matmul.
  - GCN algebra is reordered so each conv aggregates 128-dim features:
      conv1: agg(x) @ W1   (aggregate-then-transform, d_in=128 < d_out=512)
      conv2: agg(h1 @ W3)  (transform-then-aggregate, d_out=128 < d_in=512)
    with agg(z) = diag(dinv) (S + I) diag(dinv) z, dinv = 1/sqrt(deg).
  - Per conv: each core scales its node block by dinv (pre-scale), an
    AllGather builds the full bf16 node table in every core's HBM, then
    dma_gather (GPSIMD SWDGE) pulls source rows chunk-by-chunk, a 0/1
    one-hot built on the vector engine (broadcast-AP is_equal vs iota)
    scatter-reduces them on the tensor engine into PSUM per dst tile
    (transposed layout: [feat x dst]), and the dst-side dinv post-scale is
    applied on evacuation.
  - Dense layers run in feature-on-partition layout so biases fold into
    per-partition activation ops; leaky relu is max(v, 0.01 v) on DVE.
  - int16 gather indices can't span 50176 rows, so each batch issues two
    gathers: one against the table base (rows < 32768) and one against a
    +32768-row slice.

Host-side numpy does only integer graph restructuring (partition, sort,
pad, degree counts as a byproduct of sorting); every FLOP of the reference
computation runs on device.
"""
import numpy as np
import ml_dtypes
from contextlib import ExitStack
from dataclasses import dataclass

from concourse import bass, bacc, mybir, tile
from concourse import bass_utils

DT = mybir.dt
BF16 = ml_dtypes.bfloat16
HALF = 32768


@dataclass(frozen=True)
class Cfg:
    n_nodes: int = 50000
    n_cores: int = 8
    feat: int = 128          # node feature dim (must be 128)
    hid: int = 512           # conv1 output dim
    fc1: int = 256           # fc1 output dim
    slot_batch: int = 9216   # max gather slots per batch

    @property
    def nloc(self):
        return self.n_nodes // self.n_cores

    @property
    def ntile(self):
        return (self.nloc + 127) // 128

    @property
    def npad(self):
        return self.ntile * 128

    @property
    def trows(self):
        return self.npad * self.n_cores


@dataclass
class Meta:
    """Common (cross-core) slot structure + per-core host arrays."""
    lo_ch: np.ndarray        # [NT] chunks in low group per tile
    hi_ch: np.ndarray        # [NT] chunks in high group per tile
    batches: list            # list of list-of-tile-indices
    ctot: int                # total chunks
    idx_w: list              # per-core wrapped int16 [128, slots//16]
    lab_w: list              # per-core [128, ctot] bf16


def _prep(cfg: Cfg, src, dst):
    """Partition + sort edges; build the common padded slot layout."""
    NLOC, NT, NPAD = cfg.nloc, cfg.ntile, cfg.npad
    owner = dst // NLOC
    dlocal = dst - owner * NLOC
    srow = (src // NLOC) * NPAD + (src % NLOC)

    per_core = []
    for c in range(cfg.n_cores):
        sel = owner == c
        es = np.concatenate([srow[sel], c * NPAD + np.arange(NLOC, dtype=np.int64)])
        ed = np.concatenate([dlocal[sel], np.arange(NLOC, dtype=np.int64)])
        tid = ed >> 7
        lab = (ed & 127).astype(np.float32)
        hi = (es >= HALF).astype(np.int64)
        order = np.lexsort((es, hi, tid))
        per_core.append((es[order], lab[order], tid[order], hi[order]))

    # common chunk counts per tile/group = max over cores
    lo_ch = np.zeros(NT, np.int64)
    hi_ch = np.zeros(NT, np.int64)
    for es, lab, tid, hi in per_core:
        for t in range(NT):
            m = tid == t
            nlo = int(np.count_nonzero(m & (hi == 0)))
            nhi = int(np.count_nonzero(m & (hi == 1)))
            lo_ch[t] = max(lo_ch[t], (nlo + 127) // 128)
            hi_ch[t] = max(hi_ch[t], (nhi + 127) // 128)

    # batches: greedy groups of tiles
    batches, cur, cur_slots = [], [], 0
    for t in range(NT):
        ts = int(lo_ch[t] + hi_ch[t]) * 128
        if cur and cur_slots + ts > cfg.slot_batch:
            batches.append(cur)
            cur, cur_slots = [], 0
        cur.append(t)
        cur_slots += ts
    if cur:
        batches.append(cur)

    ctot = int((lo_ch + hi_ch).sum())
    tot_slots = ctot * 128

    # per-core slot streams in [batch: lo segs..., hi segs...] order
    idx_w_all, lab_w_all = [], []
    for es, lab, tid, hi in per_core:
        idx_s = np.zeros(tot_slots, np.int16)
        lab_s = np.full(tot_slots, -1.0, np.float32)
        pos = 0
        for tiles in batches:
            for grp in (0, 1):
                for t in tiles:
                    m = (tid == t) & (hi == grp)
                    e = es[m] - (HALF if grp else 0)
                    l = lab[m]
                    nch = int(lo_ch[t] if grp == 0 else hi_ch[t])
                    idx_s[pos:pos + len(e)] = e.astype(np.int16)
                    lab_s[pos:pos + len(l)] = l
                    pos += nch * 128
        assert pos == tot_slots
        # wrap idx: slot i -> [(i%16)+16r, i//16]
        iw = np.tile(idx_s.reshape(-1, 16).T, (8, 1)).copy()
        lw = lab_s.reshape(-1, 128).T.astype(BF16).copy()
        idx_w_all.append(iw)
        lab_w_all.append(lw)

    return Meta(lo_ch=lo_ch, hi_ch=hi_ch, batches=batches, ctot=ctot,
                idx_w=idx_w_all, lab_w=lab_w_all)


def _leaky(nc, out_ap, in_ap):
    nc.vector.scalar_tensor_tensor(out_ap, in_ap, 0.01, in_ap,
                                   mybir.AluOpType.mult, mybir.AluOpType.max)


def _dense_chunks(npad):
    """(offset, width) n-chunks of <=512 columns."""
    out = []
    o = 0
    while o < npad:
        w = min(512, npad - o)
        out.append((o, w))
        o += w
    return out


def build(cfg: Cfg, meta: Meta, enable_asserts=False):
    """Build + compile the SPMD program (same NEFF for all cores)."""
    NT, NPAD, TROWS, F = cfg.ntile, cfg.npad, cfg.trows, cfg.feat
    HID, FC1 = cfg.hid, cfg.fc1
    HID_T, FC1_T = HID // 128, FC1 // 128
    tot16 = meta.ctot * 128 // 16

    nc = bacc.Bacc("TRN2", target_bir_lowering=False, debug=False,
                   num_devices=cfg.n_cores, enable_asserts=enable_asserts,
                   num_swdge_queues=4)

    x_in = nc.dram_tensor("x_in", (NPAD, F), DT.float32, kind="ExternalInput")
    degc_in = nc.dram_tensor("degc_in", (128, NT), DT.float32, kind="ExternalInput")
    idx_in = nc.dram_tensor("idx_in", (128, tot16), DT.int16, kind="ExternalInput")
    lab_in = nc.dram_tensor("lab_in", (128, meta.ctot), DT.bfloat16, kind="ExternalInput")
    iota_in = nc.dram_tensor("iota_in", (128, 128), DT.bfloat16, kind="ExternalInput")
    id_in = nc.dram_tensor("id_in", (128, 128), DT.bfloat16, kind="ExternalInput")
    idf_in = nc.dram_tensor("idf_in", (128, 128), DT.float32, kind="ExternalInput")
    W1_in = nc.dram_tensor("W1_in", (F, HID), DT.float32, kind="ExternalInput")
    b1_in = nc.dram_tensor("b1_in", (HID,), DT.float32, kind="ExternalInput")
    W3_in = nc.dram_tensor("W3_in", (HID, F), DT.float32, kind="ExternalInput")
    b3_in = nc.dram_tensor("b3_in", (F,), DT.float32, kind="ExternalInput")
    f1w_in = nc.dram_tensor("f1w_in", (FC1, F), DT.float32, kind="ExternalInput")
    f1b_in = nc.dram_tensor("f1b_in", (FC1,), DT.float32, kind="ExternalInput")
    f2w_in = nc.dram_tensor("f2w_in", (1, FC1), DT.float32, kind="ExternalInput")
    f2b_in = nc.dram_tensor("f2b_in", (1,), DT.float32, kind="ExternalInput")
    out_d = nc.dram_tensor("out", (cfg.nloc,), DT.float32, kind="ExternalOutput")

    nchunks = _dense_chunks(NPAD)

    with tile.TileContext(nc) as tc:
        with ExitStack() as top:
            perm = top.enter_context(tc.tile_pool(name="perm", bufs=1))
            dram = top.enter_context(tc.tile_pool(name="dram", bufs=1, space="DRAM"))

            # ---------- constants / weights ----------
            iota_t = perm.tile([128, 128], DT.bfloat16)
            nc.sync.dma_start(iota_t[:], iota_in.ap())
            ident_t = perm.tile([128, 128], DT.bfloat16)
            nc.sync.dma_start(ident_t[:], id_in.ap())
            identf_t = perm.tile([128, 128], DT.float32)
            nc.sync.dma_start(identf_t[:], idf_in.ap())
            idx_t = perm.tile([128, tot16], DT.int16)
            nc.sync.dma_start(idx_t[:], idx_in.ap())
            lab_t = perm.tile([128, meta.ctot], DT.bfloat16)
            nc.sync.dma_start(lab_t[:], lab_in.ap())

            w1b = perm.tile([128, HID], DT.bfloat16)
            w3b = perm.tile([128, HID_T, F], DT.bfloat16)
            b1t = perm.tile([128, HID_T], DT.float32)
            b3t = perm.tile([128, 1], DT.float32)
            f1bt = perm.tile([128, FC1_T], DT.float32)
            f2bt = perm.tile([1, 1], DT.float32)
            f1wT = perm.tile([128, FC1_T, 128], DT.bfloat16)
            f2wT = perm.tile([128, FC1_T], DT.bfloat16)

            with tc.tile_pool(name="wprep", bufs=2) as wprep, \
                 tc.tile_pool(name="wprep_ps", bufs=2, space="PSUM") as wprep_ps:
                w1f = wprep.tile([128, HID], DT.float32, tag="w1stage")
                nc.sync.dma_start(w1f[:], W1_in.ap())
                nc.vector.tensor_copy(w1b[:], w1f[:])
                for k in range(HID_T):
                    w3f = wprep.tile([128, F], DT.float32, tag="w3stage")
                    nc.sync.dma_start(w3f[:], W3_in.ap()[k * 128:(k + 1) * 128, :])
                    nc.vector.tensor_copy(w3b[:, k, :], w3f[:])
                nc.sync.dma_start(b1t[:], b1_in.ap().rearrange("(c p) -> p c", p=128))
                nc.sync.dma_start(b3t[:], b3_in.ap().rearrange("(p) -> p ()"))
                nc.sync.dma_start(f1bt[:], f1b_in.ap().rearrange("(c p) -> p c", p=128))
                nc.sync.dma_start(f2bt[:], f2b_in.ap().rearrange("(u) -> u ()"))
                for k in range(FC1_T):
                    wf = wprep.tile([128, 128], DT.float32, tag="f1stage")
                    nc.sync.dma_start(wf[:], f1w_in.ap()[k * 128:(k + 1) * 128, :])
                    wb = wprep.tile([128, 128], DT.bfloat16, tag="f1stageb")
                    nc.vector.tensor_copy(wb[:], wf[:])
                    pt = wprep_ps.tile([128, 128], DT.bfloat16)
                    nc.tensor.transpose(pt[:], wb[:], ident_t[:])
                    nc.scalar.copy(f1wT[:, k, :], pt[:])
                    # fc2_w chunk k -> [128, 1]
                    wf2 = wprep.tile([128, 1], DT.float32, tag="f2stage")
                    nc.sync.dma_start(
                        wf2[:],
                        f2w_in.ap()[0:1, k * 128:(k + 1) * 128].rearrange("u (p v) -> (p u) v", p=128))
                    nc.vector.tensor_copy(f2wT[:, k:k + 1], wf2[:])

                # ---------- dinv ----------
                degc = wprep.tile([128, 128], DT.float32, tag="degc")
                nc.vector.memset(degc[:], 1.0)
                nc.sync.dma_start(degc[:, :NT], degc_in.ap())
                dinv_c = perm.tile([128, NT], DT.float32)
                rec = wprep.tile([128, 128], DT.float32, tag="rec")
                nc.vector.reciprocal(rec[:], degc[:])
                nc.scalar.activation(rec[:], rec[:], mybir.ActivationFunctionType.Sqrt)
                nc.vector.tensor_copy(dinv_c[:], rec[:, :NT])
                # transpose -> rows, ship flat to DRAM, broadcast-load
                ptd = wprep_ps.tile([128, 128], DT.float32, tag="ptd")
                nc.tensor.transpose(ptd[:], rec[:], identf_t[:])
                rows = wprep.tile([128, 128], DT.float32, tag="rows")
                nc.scalar.copy(rows[:], ptd[:])
                dflat = dram.tile([NT, 128], DT.float32)
                nc.sync.dma_start(dflat[:], rows[:NT, :])

            dinv_b = perm.tile([128, NPAD], DT.float32)
            nc.sync.dma_start(
                dinv_b[:],
                dflat[:].rearrange("t f -> () (t f)").broadcast_to([128, NPAD]))

            # ---------- tables ----------
            xs_loc = dram.tile([NPAD, F], DT.bfloat16)
            xs_tab = dram.tile([TROWS, F], DT.bfloat16, addr_space="Shared")
            zs_loc = dram.tile([NPAD, F], DT.bfloat16)
            zs_tab = dram.tile([TROWS, F], DT.bfloat16, addr_space="Shared")

            # phase A: xs_loc = dinv * x  (row layout)
            with tc.tile_pool(name="xprep", bufs=3) as xprep:
                for t in range(NT):
                    xt = xprep.tile([128, F], DT.float32, tag="xstage")
                    nc.sync.dma_start(xt[:], x_in.ap()[t * 128:(t + 1) * 128, :])
                    xst = xprep.tile([128, F], DT.bfloat16, tag="xsstage")
                    nc.scalar.activation(xst[:], xt[:], mybir.ActivationFunctionType.Copy,
                                         scale=dinv_c[:, t:t + 1])
                    nc.sync.dma_start(xs_loc[t * 128:(t + 1) * 128, :], xst[:])
            nc.gpsimd.collective_compute(
                "AllGather", mybir.AluOpType.bypass,
                replica_groups=[list(range(cfg.n_cores))],
                ins=[xs_loc.opt()], outs=[xs_tab.opt()])

            # ---------- aggregation driver ----------
            gq = [0]

            oh_stash = dram.tile([128, meta.ctot, 128], DT.bfloat16)

            def aggregate(table, evac, oh_mode):
                """Gather+scatter-matmul all batches; evac(t, psum_ap) consumes
                the [feat x dst] fp32 accumulation of tile t. oh_mode: 'build'
                = DVE is_equal + stash to DRAM, 'replay' = DMA from stash."""
                with tc.tile_pool(name="gpool", bufs=3) as gpool, \
                     tc.tile_pool(name="opool", bufs=3) as opool, \
                     tc.tile_pool(name="agg_ps", bufs=6, space="PSUM") as agg_ps:
                    c0 = 0
                    for tiles in meta.batches:
                        lo_b = int(sum(meta.lo_ch[t] for t in tiles))
                        hi_b = int(sum(meta.hi_ch[t] for t in tiles))
                        c_b = lo_b + hi_b
                        gbuf = gpool.tile([128, c_b, F], DT.bfloat16, tag="gbuf")
                        GCH = 8  # max chunks (1024 slots) per gather call
                        segs = [(c0, lo_b, table[:])]
                        if hi_b:
                            segs.append((c0 + lo_b, hi_b, table[HALF:, :]))
                        for seg_c0, seg_n, tab_ap in segs:
                            for g0 in range(0, seg_n, GCH):
                                gn = min(GCH, seg_n - g0)
                                bc = seg_c0 - c0 + g0   # buffer-local chunk
                                sl = (seg_c0 + g0) * 128
                                nc.gpsimd.dma_gather(
                                    gbuf[:, bc:bc + gn, :], tab_ap,
                                    idx_t[:, sl // 16:(sl + gn * 128) // 16],
                                    num_idxs=gn * 128, num_idxs_reg=gn * 128,
                                    elem_size=F, queue_num=gq[0] % 4)
                                gq[0] += 1
                        oh = opool.tile([128, c_b, 128], DT.bfloat16, tag="oh")
                        if oh_mode == "build":
                            nc.vector.tensor_tensor(
                                oh[:],
                                lab_t[:, c0:c0 + c_b].broadcast_to([128, c_b, 128]),
                                iota_t[:].rearrange("p d -> p () d").broadcast_to([128, c_b, 128]),
                                mybir.AluOpType.is_equal)
                            nc.sync.dma_start(oh_stash[:, c0:c0 + c_b, :], oh[:])
                        else:
                            nc.sync.dma_start(oh[:], oh_stash[:, c0:c0 + c_b, :])
                        lo_off, hi_off = 0, lo_b
                        for t in tiles:
                            chunks = ([lo_off + j for j in range(int(meta.lo_ch[t]))]
                                      + [hi_off + j for j in range(int(meta.hi_ch[t]))])
                            pt = agg_ps.tile([128, 128], DT.float32, tag="aggps")
                            for k, j in enumerate(chunks):
                                nc.tensor.matmul(pt[:], lhsT=gbuf[:, j, :], rhs=oh[:, j, :],
                                                 start=(k == 0), stop=(k == len(chunks) - 1))
                            evac(t, pt)
                            lo_off += int(meta.lo_ch[t])
                            hi_off += int(meta.hi_ch[t])
                        c0 += c_b

            # ---------- conv1 + z table ----------
            with ExitStack() as s1:
                h1pool = s1.enter_context(tc.tile_pool(name="h1pool", bufs=1))
                h1T = h1pool.tile([128, HID_T, NPAD], DT.bfloat16)

                with ExitStack() as s0:
                    c1pool = s0.enter_context(tc.tile_pool(name="c1pool", bufs=1))
                    a1T = c1pool.tile([128, NPAD], DT.bfloat16)

                    def evac1(t, pt):
                        nc.vector.tensor_mul(a1T[:, t * 128:(t + 1) * 128], pt[:],
                                             dinv_b[:, t * 128:(t + 1) * 128])

                    aggregate(xs_tab, evac1, "build")

                    # h1T[fo] = leaky(a1T.T @ W1 + b1) in [fo x n] layout
                    with tc.tile_pool(name="c1ps", bufs=3, space="PSUM") as c1ps, \
                         tc.tile_pool(name="c1tmp", bufs=3) as c1tmp:
                        for (o, w) in nchunks:
                            for fo in range(HID_T):
                                pd = c1ps.tile([128, 512], DT.float32, tag="c1ps")
                                nc.tensor.matmul(pd[:, :w], lhsT=w1b[:, fo * 128:(fo + 1) * 128],
                                                 rhs=a1T[:, o:o + w], start=True, stop=True)
                                tmp = c1tmp.tile([128, 512], DT.float32, tag="c1t")
                                nc.scalar.activation(tmp[:, :w], pd[:, :w],
                                                     mybir.ActivationFunctionType.Identity,
                                                     bias=b1t[:, fo:fo + 1])
                                _leaky(nc, h1T[:, fo, o:o + w], tmp[:, :w])

                # z2 -> zs table (scaled, transposed to row layout)
                with tc.tile_pool(name="zpool", bufs=1) as zpool, \
                     tc.tile_pool(name="zps", bufs=3, space="PSUM") as zps:
                    zsT = zpool.tile([128, NPAD], DT.bfloat16)
                    for (o, w) in nchunks:
                        pz = zps.tile([128, 512], DT.float32, tag="zps")
                        for k in range(HID_T):
                            nc.tensor.matmul(pz[:, :w], lhsT=w3b[:, k, :],
                                             rhs=h1T[:, k, o:o + w],
                                             start=(k == 0), stop=(k == HID_T - 1))
                        nc.vector.tensor_mul(zsT[:, o:o + w], pz[:, :w], dinv_b[:, o:o + w])
                    with tc.tile_pool(name="trps", bufs=3, space="PSUM") as trps, \
                         tc.tile_pool(name="trsb", bufs=3) as trsb:
                        for t in range(NT):
                            ptr = trps.tile([128, 128], DT.bfloat16, tag="trp")
                            nc.tensor.transpose(ptr[:], zsT[:, t * 128:(t + 1) * 128], ident_t[:])
                            row = trsb.tile([128, F], DT.bfloat16, tag="trs")
                            nc.scalar.copy(row[:], ptr[:])
                            nc.sync.dma_start(zs_loc[t * 128:(t + 1) * 128, :], row[:])
            nc.gpsimd.collective_compute(
                "AllGather", mybir.AluOpType.bypass,
                replica_groups=[list(range(cfg.n_cores))],
                ins=[zs_loc.opt()], outs=[zs_tab.opt()])

            # ---------- conv2 / fc ----------
            with ExitStack() as s2:
                h2pool = s2.enter_context(tc.tile_pool(name="h2pool", bufs=1))
                h2T = h2pool.tile([128, NPAD], DT.bfloat16)
                with tc.tile_pool(name="e_tmp", bufs=4) as e_tmp:

                    def evac2(t, pt):
                        sl = slice(t * 128, (t + 1) * 128)
                        v = e_tmp.tile([128, 128], DT.float32, tag="ev")
                        nc.vector.tensor_mul(v[:], pt[:], dinv_b[:, sl])
                        v2 = e_tmp.tile([128, 128], DT.float32, tag="ev2")
                        nc.scalar.activation(v2[:], v[:],
                                             mybir.ActivationFunctionType.Identity,
                                             bias=b3t[:])
                        _leaky(nc, h2T[:, sl], v2[:])

                    aggregate(zs_tab, evac2, "replay")

                with ExitStack() as s3:
                    h3pool = s3.enter_context(tc.tile_pool(name="h3pool", bufs=1))
                    h3T = h3pool.tile([128, FC1_T, NPAD], DT.bfloat16)
                    with tc.tile_pool(name="fps", bufs=3, space="PSUM") as fps, \
                         tc.tile_pool(name="ftmp", bufs=3) as ftmp:
                        for (o, w) in nchunks:
                            for k in range(FC1_T):
                                pf = fps.tile([128, 512], DT.float32, tag="fps")
                                nc.tensor.matmul(pf[:, :w], lhsT=f1wT[:, k, :],
                                                 rhs=h2T[:, o:o + w], start=True, stop=True)
                                tmp = ftmp.tile([128, 512], DT.float32, tag="ft")
                                nc.scalar.activation(tmp[:, :w], pf[:, :w],
                                                     mybir.ActivationFunctionType.Identity,
                                                     bias=f1bt[:, k:k + 1])
                                _leaky(nc, h3T[:, k, o:o + w], tmp[:, :w])

                    out_sb = h3pool.tile([1, NPAD], DT.float32)
                    with tc.tile_pool(name="gps", bufs=2, space="PSUM") as gps:
                        for (o, w) in nchunks:
                            pg = gps.tile([1, 512], DT.float32, tag="gps")
                            for k in range(FC1_T):
                                nc.tensor.matmul(pg[:, :w], lhsT=f2wT[:, k:k + 1],
                                                 rhs=h3T[:, k, o:o + w],
                                                 start=(k == 0), stop=(k == FC1_T - 1))
                            nc.scalar.activation(out_sb[:, o:o + w], pg[:, :w],
                                                 mybir.ActivationFunctionType.Identity,
                                                 bias=f2bt[:])
                    nc.sync.dma_start(out_d.ap().rearrange("(n) -> () n"), out_sb[:, :cfg.nloc])

    nc.compile()
    return nc


def make_in_maps(cfg: Cfg, meta: Meta, x, deg, W1, b1, W3, b3, fc1_w, fc1_b, fc2_w, fc2_b):
    NLOC, NT, NPAD, F = cfg.nloc, cfg.ntile, cfg.npad, cfg.feat
    iota_np = np.broadcast_to(np.arange(128, dtype=np.float32)[None, :], (128, 128)).astype(BF16).copy()
    ident_np = np.eye(128, dtype=np.float32)
    shared = dict(
        iota_in=iota_np, id_in=ident_np.astype(BF16), idf_in=ident_np,
        W1_in=np.ascontiguousarray(W1, np.float32),
        b1_in=np.ascontiguousarray(b1, np.float32),
        W3_in=np.ascontiguousarray(W3, np.float32),
        b3_in=np.ascontiguousarray(b3, np.float32),
        f1w_in=np.ascontiguousarray(fc1_w, np.float32),
        f1b_in=np.ascontiguousarray(fc1_b, np.float32),
        f2w_in=np.ascontiguousarray(fc2_w, np.float32).reshape(1, cfg.fc1),
        f2b_in=np.ascontiguousarray(fc2_b, np.float32).reshape(1),
    )
    in_maps = []
    for c in range(cfg.n_cores):
        xc = np.zeros((NPAD, F), np.float32)
        xc[:NLOC] = x[c * NLOC:(c + 1) * NLOC]
        dc = np.ones(NPAD, np.float32)
        dc[:NLOC] = deg[c * NLOC:(c + 1) * NLOC]
        degc = dc.reshape(NT, 128).T.copy()
        in_maps.append(dict(shared, x_in=xc, degc_in=degc,
                            idx_in=meta.idx_w[c], lab_in=meta.lab_w[c]))
    return in_maps


_BUILD_CACHE = {}


def kernel(x, edge_index, W1, b1, W3, b3, fc1_w, fc1_b, fc2_w, fc2_b):
    cfg = Cfg()
    x = np.asarray(x, np.float32)
    ei = np.asarray(edge_index)
    src = ei[0].astype(np.int64)
    dst = ei[1].astype(np.int64)
    deg = (np.bincount(dst, minlength=cfg.n_nodes) + 1).astype(np.float32)

    key = hash(ei.tobytes())
    if key not in _BUILD_CACHE:
        meta = _prep(cfg, src, dst)
        nc = build(cfg, meta)
        _BUILD_CACHE[key] = (meta, nc)
    meta, nc = _BUILD_CACHE[key]

    in_maps = make_in_maps(cfg, meta, x, deg, W1, b1, W3, b3, fc1_w, fc1_b, fc2_w, fc2_b)
    res = bass_utils.run_bass_kernel_spmd(nc, in_maps, core_ids=list(range(cfg.n_cores)))
    out = np.concatenate([res.results[c]["out"] for c in range(cfg.n_cores)])
    return out.astype(np.float32)
